# revision 22
# baseline (speedup 1.0000x reference)
"""Multi-head attention (B=2, S=2048, D=1024, H=16, dk=dv=64) on 8 trn2 cores.

Sharding: (batch, head-quad) -> core.  Core i handles batch i//4 and the 4
heads [4*(i%4), 4*(i%4)+4).  Each core computes its partial output
context_h @ W_O[h-slice] summed over its 4 heads; the host sums the 4
partials per batch (the "all-reduce" of the row-sharded output projection).

v2 schedule (vs the 205us baseline): the kernel is a single interleaved
stream built around keeping ScalarE's exp pipeline (the serial softmax
resource, ~1.1us per 128x1024 tile) and the PE dense simultaneously:

  - DMA: need-ordered 1MB column-block descriptors (tri, wk, xk[q0], wq,
    xq[q0], wv, xv[q0], xq[q1], xk[q1], wo, xv[q1], xq[q2], xk[q2], ...)
    so K/Q proj block 0 and the first scores/exp start ~15us earlier.
  - PSUM: sc 2x2 banks (scores), ctx 2x1 (A@V accumulators, live per pair),
    fill 2x1 (QKV proj + out-proj groups) -- so "fill" matmuls weave into
    the PE queue mid-pair instead of only at pair boundaries.
  - Causal column restriction: diagonal tasks compute scores/exp/A@V only
    for q >= 128*u (the visible columns); the mask shrinks to one 128x128
    triangle multiply per head (DVE 2x mode) instead of 128x1024.
  - Fills (K/Q proj blocks 1-3, V proj tiles, lagged out-proj tiles) are
    generators stepped ~2x per task between produce/consume so the PE never
    idles long enough to re-throttle (HAM) and ACT never starves.
  - Normalize: one merged [65,512] PSUM->SBUF copy (sums row + ctx rows)
    releases the ctx bank fast; reciprocal/broadcast/multiply off-path.

All matmuls bf16 in / fp32 accumulate; scores pairs run concurrently on
disjoint 64-row PE groups; A@V uses the ones-column trick so the softmax
denominators fall out of the same matmul.
"""

import os
import numpy as np
import ml_dtypes

import concourse.bacc as bacc
import concourse.tile as tile
import concourse.mybir as mybir
import concourse.bass_utils as bass_utils
from concourse.bass import ds

B, S, D, H, DK = 2, 2048, 1024, 16, 64
N_CORES = 8
HPC = 4            # heads per core
NCH = 8            # d-model chunks of 128
NB = 4             # query blocks of 512
BLK = 512
NT = 16            # s tiles of 128
VW = DK + 1        # V columns per head incl. ones column

DT = mybir.dt.bfloat16
NP_DT = ml_dtypes.bfloat16
F32 = mybir.dt.float32

TRACE = False
LAST_RESULTS = None

_CACHED_NC = None


def _build_program():
    nc = bacc.Bacc("TRN2", target_bir_lowering=False, debug=False,
                   enable_asserts=False, num_devices=N_CORES)

    # block-major: [q-block, partition, chunk, s-within-block] so one clean
    # 2D descriptor loads everything a 512-column proj block needs
    xq_d = nc.dram_tensor("xq_t", [NB, 128, NCH, BLK], DT, kind="ExternalInput")
    xk_d = nc.dram_tensor("xk_t", [NB, 128, NCH, BLK], DT, kind="ExternalInput")
    xv_d = nc.dram_tensor("xv_t", [NB, 128, NCH, BLK], DT, kind="ExternalInput")
    wq_d = nc.dram_tensor("wq", [128, NCH, HPC * DK], DT, kind="ExternalInput")
    wk_d = nc.dram_tensor("wk", [128, NCH, HPC * DK], DT, kind="ExternalInput")
    wv_d = nc.dram_tensor("wv", [128, NCH, HPC * DK], DT, kind="ExternalInput")
    wo_d = nc.dram_tensor("wo", [128, 2, D], DT, kind="ExternalInput")
    tri_d = nc.dram_tensor("tri01", [128, 128], DT, kind="ExternalInput")
    out_d = nc.dram_tensor("out_partial", [S, D], F32, kind="ExternalOutput")
    dbg = {}
    if os.environ.get("KDBG"):
        dbg["qt"] = nc.dram_tensor("qt_dump", [128, 2, S], DT, kind="ExternalOutput")
        dbg["kt"] = nc.dram_tensor("kt_dump", [128, 2, S], DT, kind="ExternalOutput")
        dbg["v"] = nc.dram_tensor("v_dump", [128, NT, HPC * VW], DT, kind="ExternalOutput")
        dbg["ctxt"] = nc.dram_tensor("ctxt_dump", [128, 2, S], DT, kind="ExternalOutput")

    with tile.TileContext(nc) as tc:
        _body(tc, xq_d, xk_d, xv_d, wq_d, wk_d, wv_d, wo_d, tri_d, out_d, dbg)
    nc.compile()
    return nc


def _body(tc, xq_d, xk_d, xv_d, wq_d, wk_d, wv_d, wo_d, tri_d, out_d, dbg=None):
    nc = tc.nc
    EXP = mybir.ActivationFunctionType.Exp
    CPY = mybir.ActivationFunctionType.Copy
    MUL = mybir.AluOpType.mult

    with (
        tc.tile_pool(name="consts", bufs=1) as consts,
        tc.tile_pool(name="persist", bufs=1) as persist,
        tc.tile_pool(name="xbufs", bufs=1) as xbufs,
        tc.tile_pool(name="pt", bufs=14) as pt_pool,
        tc.tile_pool(name="raw", bufs=3) as raw_pool,
        tc.tile_pool(name="small", bufs=2) as small,
        tc.tile_pool(name="osb", bufs=2) as ob_pool,
        tc.tile_pool(name="psum_sc", bufs=2, space="PSUM") as sc_pool,
        tc.tile_pool(name="psum_ctx", bufs=2, space="PSUM") as ctx_pool,
        tc.tile_pool(name="psum_fill", bufs=2, space="PSUM") as fill_pool,
    ):
        # ---- constants / persistent activations ----
        wq_sb = consts.tile([128, NCH, HPC * DK], DT)
        wk_sb = consts.tile([128, NCH, HPC * DK], DT)
        wv_sb = consts.tile([128, NCH, HPC * DK], DT)
        wo_sb = consts.tile([128, 2, D], DT)
        tri_sb = consts.tile([128, 128], DT)

        qt_sb = persist.tile([128, 2, S], DT)         # Q^T, pair-major
        kt_sb = persist.tile([128, 2, S], DT)         # K^T
        v_sb = persist.tile([128, NT, HPC * VW], DT)  # V + ones cols
        ctxt_sb = persist.tile([128, 2, S], DT)       # context^T

        xq_sb = xbufs.tile([128, NB, NCH, BLK], DT)
        xk_sb = xbufs.tile([128, NB, NCH, BLK], DT)
        xv_sb = xbufs.tile([128, NB, NCH, BLK], DT)

        # scalar-engine exp-table warmup (runs during the input DMAs)
        scr = small.tile([1, 16], F32, name="scr", tag="scr")
        scr2 = small.tile([1, 16], DT, name="scr2", tag="scr")
        nc.vector.memset(scr[:], 0.0)
        nc.scalar.activation(scr2[:], scr[:], EXP, scale=1.0)

        for hh in range(HPC):
            nc.vector.memset(v_sb[:, :, hh * VW + DK: hh * VW + DK + 1], 1.0)

        # ---- input DMAs: need-ordered 1MB block descriptors ----
        def ld_x(sb, dr, q):
            nc.sync.dma_start(sb[:, q], dr[q])

        nc.sync.dma_start(wk_sb[:], wk_d[:])
        ld_x(xk_sb, xk_d, 0)
        nc.sync.dma_start(wq_sb[:], wq_d[:])
        ld_x(xq_sb, xq_d, 0)
        nc.sync.dma_start(tri_sb[:], tri_d[:])
        nc.sync.dma_start(wv_sb[:], wv_d[:])
        ld_x(xv_sb, xv_d, 0)
        ld_x(xq_sb, xq_d, 1)
        ld_x(xk_sb, xk_d, 1)
        nc.sync.dma_start(wo_sb[:], wo_d[:])
        ld_x(xv_sb, xv_d, 1)
        ld_x(xq_sb, xq_d, 2)
        ld_x(xk_sb, xk_d, 2)
        ld_x(xv_sb, xv_d, 2)
        ld_x(xq_sb, xq_d, 3)
        ld_x(xk_sb, xk_d, 3)
        ld_x(xv_sb, xv_d, 3)

        # ---- fill generators (each yield ~= 2 N=512-class matmuls) ----
        def gen_qkproj(dst, w_sb, x_sb, blk, copy_eng):
            ps = [fill_pool.tile([128, BLK], F32, name=f"qk{p}", tag="fill")
                  for p in range(2)]
            for c in range(NCH):
                for p in range(2):
                    nc.tensor.matmul(
                        ps[p][:],
                        lhsT=w_sb[:, c, ds(128 * p, 128)],
                        rhs=x_sb[:, blk, c, :],
                        start=(c == 0), stop=(c == NCH - 1))
                if c % 2 == 1 and c < NCH - 1:
                    yield
            for p in range(2):
                dstp = dst[:, p, ds(BLK * blk, BLK)]
                if copy_eng == "scalar":
                    nc.scalar.activation(dstp, ps[p][:], CPY)
                else:
                    nc.vector.tensor_copy(dstp, ps[p][:])
            yield

        def gen_vproj(t):
            ps = fill_pool.tile([128, HPC * DK], F32, name="vps", tag="fill")
            for c in range(NCH):
                nc.tensor.matmul(
                    ps[:],
                    lhsT=xv_sb[:, t // 4, c, ds(128 * (t % 4), 128)],
                    rhs=wv_sb[:, c, :],
                    start=(c == 0), stop=(c == NCH - 1))
                if c in (2, 5):
                    yield
            dst = v_sb[:, t, :].rearrange(
                "p (hh e) -> p hh e", hh=HPC)[:, :, 0:DK]
            nc.vector.tensor_copy(dst, ps[:].rearrange(
                "p (hh e) -> p hh e", hh=HPC))
            yield

        def gen_outproj(t):
            pp = [fill_pool.tile([128, BLK], F32, name=f"pp{nb}", tag="fill")
                  for nb in range(2)]
            for cc in range(2):
                for nb in range(2):
                    nc.tensor.matmul(
                        pp[nb][:],
                        lhsT=ctxt_sb[:, cc, ds(128 * t, 128)],
                        rhs=wo_sb[:, cc, ds(512 * nb, 512)],
                        start=(cc == 0), stop=(cc == 1))
                yield
            ob = ob_pool.tile([128, D], F32, name="ob", tag="ob")
            for nb in range(2):
                nc.vector.tensor_copy(ob[:, ds(512 * nb, 512)], pp[nb][:])
            nc.sync.dma_start(out_d[ds(128 * t, 128), :], ob[:])
            yield

        # ---- attention stream ops ----
        def produce(b, hp, skt):
            u = skt - 4 * b
            qlo = 128 * u if u >= 0 else 0
            w = BLK - qlo
            sc = sc_pool.tile([128, 2, BLK], F32, name="sc", tag="sc")
            for h2 in range(2):
                nc.tensor.matmul(
                    sc[:, h2, qlo:BLK],
                    lhsT=kt_sb[ds(64 * h2, 64), hp, ds(128 * skt, 128)],
                    rhs=qt_sb[ds(64 * h2, 64), hp,
                              ds(BLK * b + qlo, w)],
                    start=True, stop=True)
            pt = pt_pool.tile([128, 2, BLK], DT, name="pt", tag="pt")
            nc.scalar.activation(pt[:, :, qlo:BLK], sc[:, :, qlo:BLK],
                                 EXP, scale=0.125)
            if u >= 0:
                for h2 in range(2):
                    nc.vector.tensor_tensor(
                        pt[:, h2, qlo:qlo + 128],
                        pt[:, h2, qlo:qlo + 128], tri_sb[:], MUL)
            return pt

        def consume(b, hp, skt, pt, ctxps):
            u = skt - 4 * b
            qlo = 128 * u if u >= 0 else 0
            last = 4 * b + 3
            for h2 in range(2):
                h = 2 * hp + h2
                if h not in ctxps:
                    ctxps[h] = ctx_pool.tile(
                        [128, BLK], F32, name=f"ctx{h2}", tag="ctx")
                nc.tensor.matmul(
                    ctxps[h][0:VW, qlo:BLK],
                    lhsT=v_sb[:, skt, ds(h * VW, VW)],
                    rhs=pt[:, h2, qlo:BLK],
                    start=(skt == 0), stop=(skt == last))

        def normalize_pair(b, hp, ctxps):
            # DVE order: sums+recip first so both gpsimd broadcasts start
            # early; raw copies and multiplies follow (shortest critical path
            # to releasing ctxt for the out-projection).
            sums, r, raw, bc = {}, {}, {}, {}
            for h2 in range(2):
                h = 2 * hp + h2
                sums[h2] = small.tile([1, BLK], F32, name="sums", tag="sums")
                nc.vector.tensor_copy(sums[h2][:], ctxps[h][ds(DK, 1), :])
                r[h2] = small.tile([1, BLK], F32, name="r", tag="r")
                nc.vector.reciprocal_approx_fast(out=r[h2][:], in_=sums[h2][:])
                bc[h2] = small.tile([64, BLK], F32, name="bc", tag="bc")
                nc.gpsimd.partition_broadcast(bc[h2][:], r[h2][:])
            for h2 in range(2):
                h = 2 * hp + h2
                raw[h2] = raw_pool.tile([DK, BLK], F32, name="raw", tag="raw")
                nc.vector.tensor_copy(raw[h2][:], ctxps[h][0:DK, :])
            for h2 in range(2):
                nc.vector.tensor_tensor(
                    ctxt_sb[ds(64 * h2, 64), hp, ds(BLK * b, BLK)],
                    raw[h2][:], bc[h2][:], MUL)

        # ---- the interleaved schedule (model-paced) ----
        # K/Q proj block 0 first (scalar-engine copies: ACT is idle here)
        for _ in gen_qkproj(kt_sb, wk_sb, xk_sb, 0, "scalar"):
            pass
        for _ in gen_qkproj(qt_sb, wq_sb, xq_sb, 0, "scalar"):
            pass

        # build-time cost model (ns) to pace fills between produce/consume
        QK_STEP, VP_STEP, OP_STEP = 520, 340, 520
        _gen_ids = {}

        def vp(t, ready):
            g = gen_vproj(t)
            _gen_ids[g] = ("vp", t)
            return (g, ready, VP_STEP)

        def qk(kind, blk, ready):
            dst, w, x = ((qt_sb, wq_sb, xq_sb) if kind == "qb"
                         else (kt_sb, wk_sb, xk_sb))
            g = gen_qkproj(dst, w, x, blk, "vector")
            _gen_ids[g] = (kind, blk)
            return (g, ready, QK_STEP)

        # (generator, est DMA-ready ns, est PE ns per step)
        fills = [
            qk("qb", 1, 23500), qk("kb", 1, 26500),
            vp(0, 21000), vp(1, 21000), vp(2, 21000), vp(3, 21000),
            qk("qb", 2, 33500), qk("kb", 2, 36500),
            vp(4, 30500), vp(5, 30500), vp(6, 30500), vp(7, 30500),
            qk("qb", 3, 42000), qk("kb", 3, 45000),
            vp(8, 39000), vp(9, 39000), vp(10, 39000), vp(11, 39000),
            vp(12, 47500), vp(13, 47500), vp(14, 47500), vp(15, 47500),
        ]
        vp_emitted = [False] * NT       # gen_vproj(t) fully stepped
        qb_emitted = [True, False, False, False]
        kb_emitted = [True, False, False, False]
        state = dict(active=None, pending_ops=[], pe=15200.0, act=17000.0,
                     held_ops=[])

        def fill_step(respect_gates=True):
            while True:
                if state["active"] is None:
                    if state["pending_ops"]:
                        state["active"] = (state["pending_ops"].pop(0),
                                           0, OP_STEP)
                    elif fills:
                        if respect_gates and state["pe"] < fills[0][1] - 1500:
                            return False
                        state["active"] = fills.pop(0)
                    else:
                        return False
                g, ready, cost = state["active"]
                try:
                    next(g)
                    state["pe"] = max(state["pe"], ready) + cost
                    return True
                except StopIteration:
                    state["active"] = None
                    kind, idx = _gen_ids.get(g, (None, None))
                    if kind == "vp":
                        vp_emitted[idx] = True
                    elif kind == "qb":
                        qb_emitted[idx] = True
                    elif kind == "kb":
                        kb_emitted[idx] = True

        pairs = [(0, 0), (0, 1), (1, 0), (1, 1),
                 (2, 0), (2, 1), (3, 0), (3, 1)]
        tasks = [(b, hp, skt) for (b, hp) in pairs for skt in range(4 * b + 4)]

        queue = []          # produced-but-unconsumed (b, hp, skt, pt)
        ctx_maps = {}

        def width(b, skt):
            u = skt - 4 * b
            return BLK - 128 * u if u >= 0 else BLK

        def try_consume(limit, drain=False):
            n = 0
            while queue and n < limit:
                b, hp, skt, pt = queue[0]
                if not vp_emitted[skt]:
                    return
                queue.pop(0)
                ctxps = ctx_maps.setdefault((b, hp), {})
                consume(b, hp, skt, pt, ctxps)
                state["pe"] += 2 * (width(b, skt) / 2.4 + 50)
                n += 1
                if skt == 4 * b + 3:    # pair complete
                    normalize_pair(b, hp, ctxps)
                    del ctx_maps[(b, hp)]
                    if hp == 1:
                        for t in range(4 * b, 4 * b + 4):
                            g = gen_outproj(t)
                            # hold block-2 out-proj as tail bridge work
                            if b == 2 and not drain:
                                state["held_ops"].append(g)
                            else:
                                state["pending_ops"].append(g)

        prev_act_end = 0.0
        for k, (b, hp, skt) in enumerate(tasks):
            # emission-order invariants (Tile only sees deps on already-
            # emitted instructions):
            #  - the qt/kt writes produce(k) reads must be emitted first
            #  - pt ring: the consume of the slot produce(k) reuses too
            while not (qb_emitted[b] and kb_emitted[skt // 4]):
                fill_step(respect_gates=False)
            while len(queue) >= 11:
                n0 = len(queue)
                try_consume(2)
                if len(queue) == n0:
                    fill_step(respect_gates=False)
            w = width(b, skt)
            # sc pool (2 bufs): produce(k) waits until exp(k-2) done
            state["pe"] = max(state["pe"], prev_act_end)
            pt = produce(b, hp, skt)
            state["pe"] += w / 2.4 + 80
            act_start = max(state["act"], state["pe"])
            new_act_end = act_start + (2 * w + 352) / 1.2
            prev_act_end = state["act"]
            state["act"] = new_act_end
            queue.append((b, hp, skt, pt))
            try_consume(2)
            # fill while the PE is ahead of the exp stream
            while state["pe"] + 600 < state["act"]:
                if not fill_step():
                    break

        state["pending_ops"].extend(state["held_ops"])
        state["held_ops"] = []
        while queue:
            try_consume(4, drain=True)
            fill_step(respect_gates=False)
        while fill_step(respect_gates=False):
            pass
        if dbg:
            nc.sync.dma_start(dbg["qt"][:], qt_sb[:])
            nc.sync.dma_start(dbg["kt"][:], kt_sb[:])
            nc.sync.dma_start(dbg["v"][:], v_sb[:])
            nc.sync.dma_start(dbg["ctxt"][:], ctxt_sb[:])


def _make_tri():
    i = np.arange(128)[:, None]
    j = np.arange(128)[None, :]
    return (i <= j).astype(NP_DT)


def _prep_core_inputs(inputs, core):
    b = core // 4
    h0 = HPC * (core % 4)
    c0, c1 = h0 * DK, (h0 + HPC) * DK
    f32 = np.float32

    def t_chunks(x):  # [S, D] -> [NB, 128, NCH, BLK] (block-major x^T)
        xt = np.asarray(x, f32).T.reshape(NCH, 128, NB, BLK)
        return np.ascontiguousarray(xt.transpose(2, 1, 0, 3)).astype(NP_DT)

    return {
        "xq_t": t_chunks(inputs["input_Q"][b]),
        "xk_t": t_chunks(inputs["input_K"][b]),
        "xv_t": t_chunks(inputs["input_V"][b]),
        "wq": np.ascontiguousarray(np.asarray(inputs["W_Q"], f32)[:, c0:c1].reshape(NCH, 128, HPC * DK).transpose(1, 0, 2)).astype(NP_DT),
        "wk": np.ascontiguousarray(np.asarray(inputs["W_K"], f32)[:, c0:c1].reshape(NCH, 128, HPC * DK).transpose(1, 0, 2)).astype(NP_DT),
        "wv": np.ascontiguousarray(np.asarray(inputs["W_V"], f32)[:, c0:c1].reshape(NCH, 128, HPC * DK).transpose(1, 0, 2)).astype(NP_DT),
        "wo": np.ascontiguousarray(np.asarray(inputs["W_O"], f32)[c0:c1, :].reshape(2, 128, D).transpose(1, 0, 2)).astype(NP_DT),
        "tri01": _make_tri(),
    }


def get_program():
    global _CACHED_NC
    if _CACHED_NC is None:
        _CACHED_NC = _build_program()
    return _CACHED_NC


def kernel(**inputs):
    global LAST_RESULTS
    nc = get_program()
    in_maps = [_prep_core_inputs(inputs, core) for core in range(N_CORES)]
    res = bass_utils.run_bass_kernel_spmd(
        nc, in_maps, core_ids=list(range(N_CORES)),
        trace=TRACE or bool(int(os.environ.get("BASS_TRACE", "0") or 0)))
    LAST_RESULTS = res
    out = np.zeros((B, S, D), np.float32)
    for core in range(N_CORES):
        out[core // 4] += res.results[core]["out_partial"]
    return out


# revision 28
# speedup vs baseline: 1.0184x; 1.0184x over previous
"""Multi-head attention (B=2, S=2048, D=1024, H=16, dk=dv=64) on 8 trn2 cores.

Sharding: (batch, head-quad) -> core.  Core i handles batch i//4 and the 4
heads [4*(i%4), 4*(i%4)+4).  Each core computes its partial output
context_h @ W_O[h-slice] summed over its 4 heads; the host sums the 4
partials per batch (the "all-reduce" of the row-sharded output projection).

v2 schedule (vs the 205us baseline): the kernel is a single interleaved
stream built around keeping ScalarE's exp pipeline (the serial softmax
resource, ~1.1us per 128x1024 tile) and the PE dense simultaneously:

  - DMA: need-ordered 1MB column-block descriptors (tri, wk, xk[q0], wq,
    xq[q0], wv, xv[q0], xq[q1], xk[q1], wo, xv[q1], xq[q2], xk[q2], ...)
    so K/Q proj block 0 and the first scores/exp start ~15us earlier.
  - PSUM: sc 2x2 banks (scores), ctx 2x1 (A@V accumulators, live per pair),
    fill 2x1 (QKV proj + out-proj groups) -- so "fill" matmuls weave into
    the PE queue mid-pair instead of only at pair boundaries.
  - Causal column restriction: diagonal tasks compute scores/exp/A@V only
    for q >= 128*u (the visible columns); the mask shrinks to one 128x128
    triangle multiply per head (DVE 2x mode) instead of 128x1024.
  - Fills (K/Q proj blocks 1-3, V proj tiles, lagged out-proj tiles) are
    generators stepped ~2x per task between produce/consume so the PE never
    idles long enough to re-throttle (HAM) and ACT never starves.
  - Normalize: one merged [65,512] PSUM->SBUF copy (sums row + ctx rows)
    releases the ctx bank fast; reciprocal/broadcast/multiply off-path.

All matmuls bf16 in / fp32 accumulate; scores pairs run concurrently on
disjoint 64-row PE groups; A@V uses the ones-column trick so the softmax
denominators fall out of the same matmul.
"""

import os
import numpy as np
import ml_dtypes

import concourse.bacc as bacc
import concourse.tile as tile
import concourse.mybir as mybir
import concourse.bass_utils as bass_utils
from concourse.bass import ds

B, S, D, H, DK = 2, 2048, 1024, 16, 64
N_CORES = 8
HPC = 4            # heads per core
NCH = 8            # d-model chunks of 128
NB = 4             # query blocks of 512
BLK = 512
NT = 16            # s tiles of 128
VW = DK + 1        # V columns per head incl. ones column

DT = mybir.dt.bfloat16
NP_DT = ml_dtypes.bfloat16
F32 = mybir.dt.float32

TRACE = False
LAST_RESULTS = None

_CACHED_NC = None


def _build_program():
    nc = bacc.Bacc("TRN2", target_bir_lowering=False, debug=False,
                   enable_asserts=False, num_devices=N_CORES)

    # block-major: [q-block, partition, chunk, s-within-block] so one clean
    # 2D descriptor loads everything a 512-column proj block needs
    xq_d = nc.dram_tensor("xq_t", [NB, 128, NCH, BLK], DT, kind="ExternalInput")
    xk_d = nc.dram_tensor("xk_t", [NB, 128, NCH, BLK], DT, kind="ExternalInput")
    xv_d = nc.dram_tensor("xv_t", [NB, 128, NCH, BLK], DT, kind="ExternalInput")
    wq_d = nc.dram_tensor("wq", [128, NCH, HPC * DK], DT, kind="ExternalInput")
    wk_d = nc.dram_tensor("wk", [128, NCH, HPC * DK], DT, kind="ExternalInput")
    wv_d = nc.dram_tensor("wv", [128, NCH, HPC * DK], DT, kind="ExternalInput")
    wo_d = nc.dram_tensor("wo", [128, 2, D], DT, kind="ExternalInput")
    tri_d = nc.dram_tensor("tri01", [128, 128], DT, kind="ExternalInput")
    out_d = nc.dram_tensor("out_partial", [S, D], F32, kind="ExternalOutput")
    dbg = {}
    if os.environ.get("KDBG"):
        dbg["qt"] = nc.dram_tensor("qt_dump", [128, 2, S], DT, kind="ExternalOutput")
        dbg["kt"] = nc.dram_tensor("kt_dump", [128, 2, S], DT, kind="ExternalOutput")
        dbg["v"] = nc.dram_tensor("v_dump", [128, NT, HPC * VW], DT, kind="ExternalOutput")
        dbg["ctxt"] = nc.dram_tensor("ctxt_dump", [128, 2, S], DT, kind="ExternalOutput")

    with tile.TileContext(nc) as tc:
        _body(tc, xq_d, xk_d, xv_d, wq_d, wk_d, wv_d, wo_d, tri_d, out_d, dbg)
    nc.compile()
    return nc


def _body(tc, xq_d, xk_d, xv_d, wq_d, wk_d, wv_d, wo_d, tri_d, out_d, dbg=None):
    nc = tc.nc
    EXP = mybir.ActivationFunctionType.Exp
    CPY = mybir.ActivationFunctionType.Copy
    MUL = mybir.AluOpType.mult

    with (
        tc.tile_pool(name="consts", bufs=1) as consts,
        tc.tile_pool(name="persist", bufs=1) as persist,
        tc.tile_pool(name="xbufs", bufs=1) as xbufs,
        tc.tile_pool(name="pt", bufs=14) as pt_pool,
        tc.tile_pool(name="raw", bufs=3) as raw_pool,
        tc.tile_pool(name="small", bufs=2) as small,
        tc.tile_pool(name="osb", bufs=2) as ob_pool,
        tc.tile_pool(name="psum_sc", bufs=2, space="PSUM") as sc_pool,
        tc.tile_pool(name="psum_ctx", bufs=2, space="PSUM") as ctx_pool,
        tc.tile_pool(name="psum_fill", bufs=2, space="PSUM") as fill_pool,
    ):
        # ---- constants / persistent activations ----
        wq_sb = consts.tile([128, NCH, HPC * DK], DT)
        wk_sb = consts.tile([128, NCH, HPC * DK], DT)
        wv_sb = consts.tile([128, NCH, HPC * DK], DT)
        wo_sb = consts.tile([128, 2, D], DT)
        tri_sb = consts.tile([128, 128], DT)

        qt_sb = persist.tile([128, 2, S], DT)         # Q^T, pair-major
        kt_sb = persist.tile([128, 2, S], DT)         # K^T
        v_sb = persist.tile([128, NT, HPC * VW], DT)  # V + ones cols
        ctxt_sb = persist.tile([128, 2, S], DT)       # context^T

        xq_sb = xbufs.tile([128, NB, NCH, BLK], DT)
        xk_sb = xbufs.tile([128, NB, NCH, BLK], DT)
        xv_sb = xbufs.tile([128, NB, NCH, BLK], DT)

        # scalar-engine exp-table warmup (runs during the input DMAs)
        scr = small.tile([1, 16], F32, name="scr", tag="scr")
        scr2 = small.tile([1, 16], DT, name="scr2", tag="scr")
        nc.vector.memset(scr[:], 0.0)
        nc.scalar.activation(scr2[:], scr[:], EXP, scale=1.0)

        for hh in range(HPC):
            nc.vector.memset(v_sb[:, :, hh * VW + DK: hh * VW + DK + 1], 1.0)

        # ---- input DMAs: need-ordered 1MB block descriptors ----
        def ld_x(sb, dr, q):
            nc.sync.dma_start(sb[:, q], dr[q])

        nc.sync.dma_start(wk_sb[:], wk_d[:])
        nc.sync.dma_start(xk_sb[:, 0, 0:4], xk_d[0][:, 0:4])
        nc.sync.dma_start(xk_sb[:, 0, 4:8], xk_d[0][:, 4:8])
        nc.sync.dma_start(wq_sb[:], wq_d[:])
        nc.sync.dma_start(xq_sb[:, 0, 0:4], xq_d[0][:, 0:4])
        nc.sync.dma_start(xq_sb[:, 0, 4:8], xq_d[0][:, 4:8])
        nc.sync.dma_start(tri_sb[:], tri_d[:])
        nc.sync.dma_start(wv_sb[:], wv_d[:])
        ld_x(xv_sb, xv_d, 0)
        ld_x(xq_sb, xq_d, 1)
        ld_x(xk_sb, xk_d, 1)
        nc.sync.dma_start(wo_sb[:], wo_d[:])
        ld_x(xv_sb, xv_d, 1)
        ld_x(xq_sb, xq_d, 2)
        ld_x(xk_sb, xk_d, 2)
        ld_x(xv_sb, xv_d, 2)
        ld_x(xq_sb, xq_d, 3)
        ld_x(xk_sb, xk_d, 3)
        ld_x(xv_sb, xv_d, 3)

        # ---- fill generators (each yield ~= 2 N=512-class matmuls) ----
        def gen_qkproj(dst, w_sb, x_sb, blk, copy_eng):
            ps = [fill_pool.tile([128, BLK], F32, name=f"qk{p}", tag="fill")
                  for p in range(2)]
            for c in range(NCH):
                for p in range(2):
                    nc.tensor.matmul(
                        ps[p][:],
                        lhsT=w_sb[:, c, ds(128 * p, 128)],
                        rhs=x_sb[:, blk, c, :],
                        start=(c == 0), stop=(c == NCH - 1))
                if c % 2 == 1 and c < NCH - 1:
                    yield
            for p in range(2):
                dstp = dst[:, p, ds(BLK * blk, BLK)]
                if copy_eng == "scalar":
                    nc.scalar.activation(dstp, ps[p][:], CPY)
                else:
                    nc.vector.tensor_copy(dstp, ps[p][:])
            yield

        def gen_vproj(t):
            ps = fill_pool.tile([128, HPC * DK], F32, name="vps", tag="fill")
            for c in range(NCH):
                nc.tensor.matmul(
                    ps[:],
                    lhsT=xv_sb[:, t // 4, c, ds(128 * (t % 4), 128)],
                    rhs=wv_sb[:, c, :],
                    start=(c == 0), stop=(c == NCH - 1))
                if c in (2, 5):
                    yield
            dst = v_sb[:, t, :].rearrange(
                "p (hh e) -> p hh e", hh=HPC)[:, :, 0:DK]
            nc.vector.tensor_copy(dst, ps[:].rearrange(
                "p (hh e) -> p hh e", hh=HPC))
            yield

        def gen_outproj(t):
            pp = [fill_pool.tile([128, BLK], F32, name=f"pp{nb}", tag="fill")
                  for nb in range(2)]
            for cc in range(2):
                for nb in range(2):
                    nc.tensor.matmul(
                        pp[nb][:],
                        lhsT=ctxt_sb[:, cc, ds(128 * t, 128)],
                        rhs=wo_sb[:, cc, ds(512 * nb, 512)],
                        start=(cc == 0), stop=(cc == 1))
                yield
            ob = ob_pool.tile([128, D], F32, name="ob", tag="ob")
            for nb in range(2):
                nc.vector.tensor_copy(ob[:, ds(512 * nb, 512)], pp[nb][:])
            nc.sync.dma_start(out_d[ds(128 * t, 128), :], ob[:])
            yield

        # ---- attention stream ops ----
        def produce(b, hp, skt):
            u = skt - 4 * b
            qlo = 128 * u if u >= 0 else 0
            w = BLK - qlo
            sc = sc_pool.tile([128, 2, BLK], F32, name="sc", tag="sc")
            for h2 in range(2):
                nc.tensor.matmul(
                    sc[:, h2, qlo:BLK],
                    lhsT=kt_sb[ds(64 * h2, 64), hp, ds(128 * skt, 128)],
                    rhs=qt_sb[ds(64 * h2, 64), hp,
                              ds(BLK * b + qlo, w)],
                    start=True, stop=True)
            pt = pt_pool.tile([128, 2, BLK], DT, name="pt", tag="pt")
            nc.scalar.activation(pt[:, :, qlo:BLK], sc[:, :, qlo:BLK],
                                 EXP, scale=0.125)
            if u >= 0:
                for h2 in range(2):
                    nc.vector.tensor_tensor(
                        pt[:, h2, qlo:qlo + 128],
                        pt[:, h2, qlo:qlo + 128], tri_sb[:], MUL)
            return pt

        def consume(b, hp, skt, pt, ctxps):
            u = skt - 4 * b
            qlo = 128 * u if u >= 0 else 0
            last = 4 * b + 3
            for h2 in range(2):
                h = 2 * hp + h2
                if h not in ctxps:
                    ctxps[h] = ctx_pool.tile(
                        [128, BLK], F32, name=f"ctx{h2}", tag="ctx")
                nc.tensor.matmul(
                    ctxps[h][0:VW, qlo:BLK],
                    lhsT=v_sb[:, skt, ds(h * VW, VW)],
                    rhs=pt[:, h2, qlo:BLK],
                    start=(skt == 0), stop=(skt == last))

        def normalize_pair(b, hp, ctxps):
            # DVE order: sums+recip first so both gpsimd broadcasts start
            # early; raw copies and multiplies follow (shortest critical path
            # to releasing ctxt for the out-projection).
            sums, r, raw, bc = {}, {}, {}, {}
            for h2 in range(2):
                h = 2 * hp + h2
                sums[h2] = small.tile([1, BLK], F32, name="sums", tag="sums")
                nc.vector.tensor_copy(sums[h2][:], ctxps[h][ds(DK, 1), :])
                r[h2] = small.tile([1, BLK], F32, name="r", tag="r")
                nc.vector.reciprocal_approx_fast(out=r[h2][:], in_=sums[h2][:])
                bc[h2] = small.tile([64, BLK], F32, name="bc", tag="bc")
                nc.gpsimd.partition_broadcast(bc[h2][:], r[h2][:])
            for h2 in range(2):
                h = 2 * hp + h2
                raw[h2] = raw_pool.tile([DK, BLK], F32, name="raw", tag="raw")
                nc.vector.tensor_copy(raw[h2][:], ctxps[h][0:DK, :])
            for h2 in range(2):
                nc.vector.tensor_tensor(
                    ctxt_sb[ds(64 * h2, 64), hp, ds(BLK * b, BLK)],
                    raw[h2][:], bc[h2][:], MUL)

        # ---- the interleaved schedule (model-paced) ----
        # K/Q proj block 0 first (scalar-engine copies: ACT is idle here)
        for _ in gen_qkproj(kt_sb, wk_sb, xk_sb, 0, "scalar"):
            pass
        for _ in gen_qkproj(qt_sb, wq_sb, xq_sb, 0, "scalar"):
            pass

        # build-time cost model (ns) to pace fills between produce/consume
        QK_STEP, VP_STEP, OP_STEP = 520, 340, 520
        _gen_ids = {}

        def vp(t, ready):
            g = gen_vproj(t)
            _gen_ids[g] = ("vp", t)
            return (g, ready, VP_STEP)

        def qk(kind, blk, ready):
            dst, w, x = ((qt_sb, wq_sb, xq_sb) if kind == "qb"
                         else (kt_sb, wk_sb, xk_sb))
            g = gen_qkproj(dst, w, x, blk, "vector")
            _gen_ids[g] = (kind, blk)
            return (g, ready, QK_STEP)

        # (generator, est DMA-ready ns, est PE ns per step)
        fills = [
            qk("qb", 1, 23500), qk("kb", 1, 26500),
            vp(0, 21000), vp(1, 21000), vp(2, 21000), vp(3, 21000),
            qk("qb", 2, 33500), qk("kb", 2, 36500),
            vp(4, 30500), vp(5, 30500), vp(6, 30500), vp(7, 30500),
            qk("qb", 3, 42000), qk("kb", 3, 45000),
            vp(8, 39000), vp(9, 39000), vp(10, 39000), vp(11, 39000),
            vp(12, 47500), vp(13, 47500), vp(14, 47500), vp(15, 47500),
        ]
        vp_emitted = [False] * NT       # gen_vproj(t) fully stepped
        qb_emitted = [True, False, False, False]
        kb_emitted = [True, False, False, False]
        state = dict(active=None, pending_ops=[], pe=15000.0, act=15300.0,
                     held_ops=[], steps=0)

        def fill_step(respect_gates=True):
            while True:
                if state["active"] is None:
                    if state["pending_ops"]:
                        state["active"] = (state["pending_ops"].pop(0),
                                           0, OP_STEP)
                    elif fills:
                        if respect_gates and state["pe"] < fills[0][1] - 1500:
                            return False
                        state["active"] = fills.pop(0)
                    else:
                        return False
                g, ready, cost = state["active"]
                try:
                    next(g)
                    state["pe"] = max(state["pe"], ready) + cost
                    state["steps"] += 1
                    return True
                except StopIteration:
                    state["active"] = None
                    kind, idx = _gen_ids.get(g, (None, None))
                    if kind == "vp":
                        vp_emitted[idx] = True
                    elif kind == "qb":
                        qb_emitted[idx] = True
                    elif kind == "kb":
                        kb_emitted[idx] = True

        pairs = [(0, 0), (0, 1), (1, 0), (1, 1),
                 (2, 0), (2, 1), (3, 0), (3, 1)]
        tasks = [(b, hp, skt) for (b, hp) in pairs for skt in range(4 * b + 4)]

        queue = []          # produced-but-unconsumed (b, hp, skt, pt)
        ctx_maps = {}

        def width(b, skt):
            u = skt - 4 * b
            return BLK - 128 * u if u >= 0 else BLK

        def try_consume(limit, drain=False):
            n = 0
            while queue and n < limit:
                b, hp, skt, pt = queue[0]
                if not vp_emitted[skt]:
                    return
                queue.pop(0)
                ctxps = ctx_maps.setdefault((b, hp), {})
                consume(b, hp, skt, pt, ctxps)
                state["pe"] += 2 * (width(b, skt) / 2.4 + 50)
                n += 1
                if skt == 4 * b + 3:    # pair complete
                    normalize_pair(b, hp, ctxps)
                    del ctx_maps[(b, hp)]
                    if hp == 1:
                        for t in range(4 * b, 4 * b + 4):
                            g = gen_outproj(t)
                            # hold block-2 out-proj as tail-bridge work
                            if b == 2 and not drain:
                                state["held_ops"].append(g)
                            else:
                                state["pending_ops"].append(g)
                    if (b, hp) == (3, 0):
                        # release half the bridge before the last pair's
                        # consumes hit the PE queue
                        state["pending_ops"].extend(state["held_ops"][:2])
                        state["held_ops"] = state["held_ops"][2:]

        prev_act_end = 0.0
        for k, (b, hp, skt) in enumerate(tasks):
            # emission-order invariants (Tile only sees deps on already-
            # emitted instructions):
            #  - the qt/kt writes produce(k) reads must be emitted first
            #  - pt ring: the consume of the slot produce(k) reuses too
            while not (qb_emitted[b] and kb_emitted[skt // 4]):
                fill_step(respect_gates=False)
            while len(queue) >= 11:
                n0 = len(queue)
                try_consume(2)
                if len(queue) == n0:
                    fill_step(respect_gates=False)
            w = width(b, skt)
            # sc pool (2 bufs): produce(k) waits until exp(k-2) done
            state["pe"] = max(state["pe"], prev_act_end)
            pt = produce(b, hp, skt)
            state["pe"] += w / 2.4 + 80
            act_start = max(state["act"], state["pe"])
            new_act_end = act_start + (2 * w + 352) / 1.2
            prev_act_end = state["act"]
            state["act"] = new_act_end
            queue.append((b, hp, skt, pt))
            try_consume(2)
            # fill while the PE is ahead of the exp stream, with a minimum
            # progress floor so fills never pile up into forced blocks
            while state["pe"] + 600 < state["act"]:
                if not fill_step():
                    break
            while k >= 5 and state["steps"] < 2 * (k - 4):
                if not fill_step():
                    break

        # drain: the held bridge out-projs interleave with the remaining
        # consumes so the PE has work while the exp tail and the last
        # normalize chain complete; OP12-15 (emitted at (3,1) completion)
        # land behind them in the queue
        state["pending_ops"].extend(state["held_ops"])
        state["held_ops"] = []
        while queue:
            try_consume(2, drain=True)
            fill_step(respect_gates=False)
        while fill_step(respect_gates=False):
            pass
        if dbg:
            nc.sync.dma_start(dbg["qt"][:], qt_sb[:])
            nc.sync.dma_start(dbg["kt"][:], kt_sb[:])
            nc.sync.dma_start(dbg["v"][:], v_sb[:])
            nc.sync.dma_start(dbg["ctxt"][:], ctxt_sb[:])


def _make_tri():
    i = np.arange(128)[:, None]
    j = np.arange(128)[None, :]
    return (i <= j).astype(NP_DT)


def _prep_core_inputs(inputs, core):
    b = core // 4
    h0 = HPC * (core % 4)
    c0, c1 = h0 * DK, (h0 + HPC) * DK
    f32 = np.float32

    def t_chunks(x):  # [S, D] -> [NB, 128, NCH, BLK] (block-major x^T)
        xt = np.asarray(x, f32).T.reshape(NCH, 128, NB, BLK)
        return np.ascontiguousarray(xt.transpose(2, 1, 0, 3)).astype(NP_DT)

    return {
        "xq_t": t_chunks(inputs["input_Q"][b]),
        "xk_t": t_chunks(inputs["input_K"][b]),
        "xv_t": t_chunks(inputs["input_V"][b]),
        "wq": np.ascontiguousarray(np.asarray(inputs["W_Q"], f32)[:, c0:c1].reshape(NCH, 128, HPC * DK).transpose(1, 0, 2)).astype(NP_DT),
        "wk": np.ascontiguousarray(np.asarray(inputs["W_K"], f32)[:, c0:c1].reshape(NCH, 128, HPC * DK).transpose(1, 0, 2)).astype(NP_DT),
        "wv": np.ascontiguousarray(np.asarray(inputs["W_V"], f32)[:, c0:c1].reshape(NCH, 128, HPC * DK).transpose(1, 0, 2)).astype(NP_DT),
        "wo": np.ascontiguousarray(np.asarray(inputs["W_O"], f32)[c0:c1, :].reshape(2, 128, D).transpose(1, 0, 2)).astype(NP_DT),
        "tri01": _make_tri(),
    }


def get_program():
    global _CACHED_NC
    if _CACHED_NC is None:
        _CACHED_NC = _build_program()
    return _CACHED_NC


def kernel(**inputs):
    global LAST_RESULTS
    nc = get_program()
    in_maps = [_prep_core_inputs(inputs, core) for core in range(N_CORES)]
    res = bass_utils.run_bass_kernel_spmd(
        nc, in_maps, core_ids=list(range(N_CORES)),
        trace=TRACE or bool(int(os.environ.get("BASS_TRACE", "0") or 0)))
    LAST_RESULTS = res
    out = np.zeros((B, S, D), np.float32)
    for core in range(N_CORES):
        out[core // 4] += res.results[core]["out_partial"]
    return out


# revision 33
# speedup vs baseline: 1.0566x; 1.0376x over previous
"""Multi-head attention (B=2, S=2048, D=1024, H=16, dk=dv=64) on 8 trn2 cores.

Sharding: (batch, head-quad) -> core.  Core i handles batch i//4 and the 4
heads [4*(i%4), 4*(i%4)+4).  Each core computes its partial output
context_h @ W_O[h-slice] summed over its 4 heads; the host sums the 4
partials per batch (the "all-reduce" of the row-sharded output projection).

v2 schedule (vs the 205us baseline): the kernel is a single interleaved
stream built around keeping ScalarE's exp pipeline (the serial softmax
resource, ~1.1us per 128x1024 tile) and the PE dense simultaneously:

  - DMA: need-ordered 1MB column-block descriptors (tri, wk, xk[q0], wq,
    xq[q0], wv, xv[q0], xq[q1], xk[q1], wo, xv[q1], xq[q2], xk[q2], ...)
    so K/Q proj block 0 and the first scores/exp start ~15us earlier.
  - PSUM: sc 2x2 banks (scores), ctx 2x1 (A@V accumulators, live per pair),
    fill 2x1 (QKV proj + out-proj groups) -- so "fill" matmuls weave into
    the PE queue mid-pair instead of only at pair boundaries.
  - Causal column restriction: diagonal tasks compute scores/exp/A@V only
    for q >= 128*u (the visible columns); the mask shrinks to one 128x128
    triangle multiply per head (DVE 2x mode) instead of 128x1024.
  - Fills (K/Q proj blocks 1-3, V proj tiles, lagged out-proj tiles) are
    generators stepped ~2x per task between produce/consume so the PE never
    idles long enough to re-throttle (HAM) and ACT never starves.
  - Normalize: one merged [65,512] PSUM->SBUF copy (sums row + ctx rows)
    releases the ctx bank fast; reciprocal/broadcast/multiply off-path.

All matmuls bf16 in / fp32 accumulate; scores pairs run concurrently on
disjoint 64-row PE groups; A@V uses the ones-column trick so the softmax
denominators fall out of the same matmul.
"""

import os
import numpy as np
import ml_dtypes

import concourse.bacc as bacc
import concourse.tile as tile
import concourse.mybir as mybir
import concourse.bass_utils as bass_utils
from concourse.bass import ds

B, S, D, H, DK = 2, 2048, 1024, 16, 64
N_CORES = 8
HPC = 4            # heads per core
NCH = 8            # d-model chunks of 128
NB = 4             # query blocks of 512
BLK = 512
NT = 16            # s tiles of 128
VW = DK + 1        # V columns per head incl. ones column

DT = mybir.dt.bfloat16
NP_DT = ml_dtypes.bfloat16
F32 = mybir.dt.float32

TRACE = False
LAST_RESULTS = None

_CACHED_NC = None


def _build_program():
    nc = bacc.Bacc("TRN2", target_bir_lowering=False, debug=False,
                   enable_asserts=False, num_devices=N_CORES)

    # block-major: [q-block, partition, chunk, s-within-block] so one clean
    # 2D descriptor loads everything a 512-column proj block needs
    xq_d = nc.dram_tensor("xq_t", [NB, 128, NCH, BLK], DT, kind="ExternalInput")
    xk_d = nc.dram_tensor("xk_t", [NB, 128, NCH, BLK], DT, kind="ExternalInput")
    xv_d = nc.dram_tensor("xv_t", [NB, 128, NCH, BLK], DT, kind="ExternalInput")
    wq_d = nc.dram_tensor("wq", [128, NCH, HPC * DK], DT, kind="ExternalInput")
    wk_d = nc.dram_tensor("wk", [128, NCH, HPC * DK], DT, kind="ExternalInput")
    wv_d = nc.dram_tensor("wv", [128, NCH, HPC * DK], DT, kind="ExternalInput")
    wo_d = nc.dram_tensor("wo", [128, 2, D], DT, kind="ExternalInput")
    tri_d = nc.dram_tensor("tri01", [128, 128], DT, kind="ExternalInput")
    out_d = nc.dram_tensor("out_partial", [S, D], F32, kind="ExternalOutput")
    dbg = {}
    if os.environ.get("KDBG"):
        dbg["qt"] = nc.dram_tensor("qt_dump", [128, 2, S], DT, kind="ExternalOutput")
        dbg["kt"] = nc.dram_tensor("kt_dump", [128, 2, S], DT, kind="ExternalOutput")
        dbg["v"] = nc.dram_tensor("v_dump", [128, NT, HPC * VW], DT, kind="ExternalOutput")
        dbg["ctxt"] = nc.dram_tensor("ctxt_dump", [128, 2, S], DT, kind="ExternalOutput")

    with tile.TileContext(nc) as tc:
        _body(tc, xq_d, xk_d, xv_d, wq_d, wk_d, wv_d, wo_d, tri_d, out_d, dbg)
    nc.compile()
    return nc


def _body(tc, xq_d, xk_d, xv_d, wq_d, wk_d, wv_d, wo_d, tri_d, out_d, dbg=None):
    nc = tc.nc
    EXP = mybir.ActivationFunctionType.Exp
    CPY = mybir.ActivationFunctionType.Copy
    MUL = mybir.AluOpType.mult

    with (
        tc.tile_pool(name="consts", bufs=1) as consts,
        tc.tile_pool(name="persist", bufs=1) as persist,
        tc.tile_pool(name="xbufs", bufs=1) as xbufs,
        tc.tile_pool(name="pt", bufs=14) as pt_pool,
        tc.tile_pool(name="raw", bufs=3) as raw_pool,
        tc.tile_pool(name="small", bufs=2) as small,
        tc.tile_pool(name="osb", bufs=2) as ob_pool,
        tc.tile_pool(name="psum_sc", bufs=2, space="PSUM") as sc_pool,
        tc.tile_pool(name="psum_ctx", bufs=2, space="PSUM") as ctx_pool,
        tc.tile_pool(name="psum_fill", bufs=2, space="PSUM") as fill_pool,
    ):
        # ---- constants / persistent activations ----
        wq_sb = consts.tile([128, NCH, HPC * DK], DT)
        wk_sb = consts.tile([128, NCH, HPC * DK], DT)
        wv_sb = consts.tile([128, NCH, HPC * DK], DT)
        wo_sb = consts.tile([128, 2, D], DT)
        tri_sb = consts.tile([128, 128], DT)

        qt_sb = persist.tile([128, 2, S], DT)         # Q^T, pair-major
        kt_sb = persist.tile([128, 2, S], DT)         # K^T
        v_sb = persist.tile([128, NT, HPC * VW], DT)  # V + ones cols
        ctxt_sb = persist.tile([128, 2, S], DT)       # context^T

        xq_sb = xbufs.tile([128, NB, NCH, BLK], DT)
        xk_sb = xbufs.tile([128, NB, NCH, BLK], DT)
        xv_sb = xbufs.tile([128, NB, NCH, BLK], DT)

        # scalar-engine exp-table warmup (runs during the input DMAs)
        scr = small.tile([1, 16], F32, name="scr", tag="scr")
        scr2 = small.tile([1, 16], DT, name="scr2", tag="scr")
        nc.vector.memset(scr[:], 0.0)
        nc.scalar.activation(scr2[:], scr[:], EXP, scale=1.0)

        for hh in range(HPC):
            nc.vector.memset(v_sb[:, :, hh * VW + DK: hh * VW + DK + 1], 1.0)

        # ---- input DMAs: need-ordered 1MB block descriptors ----
        def ld_x(sb, dr, q):
            nc.sync.dma_start(sb[:, q], dr[q])

        # K path on the sync queue, Q path on the gpsimd queue: descriptor
        # issue is ~1.5us each and serializes per queue, so split it
        nc.sync.dma_start(wk_sb[:], wk_d[:])
        nc.sync.dma_start(xk_sb[:, 0, 0:4], xk_d[0][:, 0:4])
        nc.sync.dma_start(xk_sb[:, 0, 4:8], xk_d[0][:, 4:8])
        nc.gpsimd.dma_start(wq_sb[:], wq_d[:])
        nc.gpsimd.dma_start(xq_sb[:, 0, 0:4], xq_d[0][:, 0:4])
        nc.gpsimd.dma_start(xq_sb[:, 0, 4:8], xq_d[0][:, 4:8])
        nc.gpsimd.dma_start(tri_sb[:], tri_d[:])
        nc.sync.dma_start(wv_sb[:], wv_d[:])
        ld_x(xv_sb, xv_d, 0)
        ld_x(xq_sb, xq_d, 1)
        ld_x(xk_sb, xk_d, 1)
        nc.sync.dma_start(wo_sb[:], wo_d[:])
        ld_x(xv_sb, xv_d, 1)
        ld_x(xq_sb, xq_d, 2)
        ld_x(xk_sb, xk_d, 2)
        ld_x(xv_sb, xv_d, 2)
        ld_x(xq_sb, xq_d, 3)
        ld_x(xk_sb, xk_d, 3)
        ld_x(xv_sb, xv_d, 3)

        # ---- fill generators (each yield ~= 2 N=512-class matmuls) ----
        def gen_qkproj(dst, w_sb, x_sb, blk, copy_eng):
            ps = [fill_pool.tile([128, BLK], F32, name=f"qk{p}", tag="fill")
                  for p in range(2)]
            for c in range(NCH):
                for p in range(2):
                    nc.tensor.matmul(
                        ps[p][:],
                        lhsT=w_sb[:, c, ds(128 * p, 128)],
                        rhs=x_sb[:, blk, c, :],
                        start=(c == 0), stop=(c == NCH - 1))
                if c % 2 == 1 and c < NCH - 1:
                    yield
            for p in range(2):
                dstp = dst[:, p, ds(BLK * blk, BLK)]
                if copy_eng == "scalar":
                    nc.scalar.activation(dstp, ps[p][:], CPY)
                else:
                    nc.vector.tensor_copy(dstp, ps[p][:])
            yield

        def gen_vproj(t):
            ps = fill_pool.tile([128, HPC * DK], F32, name="vps", tag="fill")
            for c in range(NCH):
                nc.tensor.matmul(
                    ps[:],
                    lhsT=xv_sb[:, t // 4, c, ds(128 * (t % 4), 128)],
                    rhs=wv_sb[:, c, :],
                    start=(c == 0), stop=(c == NCH - 1))
                if c in (2, 5):
                    yield
            dst = v_sb[:, t, :].rearrange(
                "p (hh e) -> p hh e", hh=HPC)[:, :, 0:DK]
            nc.vector.tensor_copy(dst, ps[:].rearrange(
                "p (hh e) -> p hh e", hh=HPC))
            yield

        def gen_outproj(t, copy_eng="vector"):
            pp = [fill_pool.tile([128, BLK], F32, name=f"pp{nb}", tag="fill")
                  for nb in range(2)]
            for cc in range(2):
                for nb in range(2):
                    nc.tensor.matmul(
                        pp[nb][:],
                        lhsT=ctxt_sb[:, cc, ds(128 * t, 128)],
                        rhs=wo_sb[:, cc, ds(512 * nb, 512)],
                        start=(cc == 0), stop=(cc == 1))
                yield
            ob = ob_pool.tile([128, D], F32, name="ob", tag="ob")
            for nb in range(2):
                dst = ob[:, ds(512 * nb, 512)]
                if copy_eng == "scalar":
                    nc.scalar.activation(dst, pp[nb][:], CPY)
                else:
                    nc.vector.tensor_copy(dst, pp[nb][:])
            nc.sync.dma_start(out_d[ds(128 * t, 128), :], ob[:])
            yield

        # ---- attention stream ops ----
        def produce(b, hp, skt):
            u = skt - 4 * b
            qlo = 128 * u if u >= 0 else 0
            w = BLK - qlo
            sc = sc_pool.tile([128, 2, BLK], F32, name="sc", tag="sc")
            for h2 in range(2):
                nc.tensor.matmul(
                    sc[:, h2, qlo:BLK],
                    lhsT=kt_sb[ds(64 * h2, 64), hp, ds(128 * skt, 128)],
                    rhs=qt_sb[ds(64 * h2, 64), hp,
                              ds(BLK * b + qlo, w)],
                    start=True, stop=True)
            pt = pt_pool.tile([128, 2, BLK], DT, name="pt", tag="pt")
            nc.scalar.activation(pt[:, :, qlo:BLK], sc[:, :, qlo:BLK],
                                 EXP, scale=0.125)
            if u >= 0:
                for h2 in range(2):
                    nc.vector.tensor_tensor(
                        pt[:, h2, qlo:qlo + 128],
                        pt[:, h2, qlo:qlo + 128], tri_sb[:], MUL)
            return pt

        def consume(b, hp, skt, pt, ctxps):
            u = skt - 4 * b
            qlo = 128 * u if u >= 0 else 0
            last = 4 * b + 3
            for h2 in range(2):
                h = 2 * hp + h2
                if h not in ctxps:
                    ctxps[h] = ctx_pool.tile(
                        [128, BLK], F32, name=f"ctx{h2}", tag="ctx")
                nc.tensor.matmul(
                    ctxps[h][0:VW, qlo:BLK],
                    lhsT=v_sb[:, skt, ds(h * VW, VW)],
                    rhs=pt[:, h2, qlo:BLK],
                    start=(skt == 0), stop=(skt == last))

        def normalize_pair(b, hp, ctxps):
            # DVE order: sums+recip first so both gpsimd broadcasts start
            # early; raw copies and multiplies follow (shortest critical path
            # to releasing ctxt for the out-projection).
            sums, r, raw, bc = {}, {}, {}, {}
            for h2 in range(2):
                h = 2 * hp + h2
                sums[h2] = small.tile([1, BLK], F32, name="sums", tag="sums")
                nc.vector.tensor_copy(sums[h2][:], ctxps[h][ds(DK, 1), :])
                r[h2] = small.tile([1, BLK], F32, name="r", tag="r")
                nc.vector.reciprocal_approx_fast(out=r[h2][:], in_=sums[h2][:])
                bc[h2] = small.tile([64, BLK], F32, name="bc", tag="bc")
                nc.gpsimd.partition_broadcast(bc[h2][:], r[h2][:])
            for h2 in range(2):
                h = 2 * hp + h2
                raw[h2] = raw_pool.tile([DK, BLK], F32, name="raw", tag="raw")
                nc.vector.tensor_copy(raw[h2][:], ctxps[h][0:DK, :])
            for h2 in range(2):
                nc.vector.tensor_tensor(
                    ctxt_sb[ds(64 * h2, 64), hp, ds(BLK * b, BLK)],
                    raw[h2][:], bc[h2][:], MUL)

        # ---- the interleaved schedule (model-paced) ----
        # K/Q proj block 0 first (scalar-engine copies: ACT is idle here)
        for _ in gen_qkproj(kt_sb, wk_sb, xk_sb, 0, "scalar"):
            pass
        for _ in gen_qkproj(qt_sb, wq_sb, xq_sb, 0, "scalar"):
            pass

        # build-time cost model (ns) to pace fills between produce/consume
        QK_STEP, VP_STEP, OP_STEP = 520, 340, 520
        _gen_ids = {}

        def vp(t, ready):
            g = gen_vproj(t)
            _gen_ids[g] = ("vp", t)
            return (g, ready, VP_STEP)

        def qk(kind, blk, ready):
            dst, w, x = ((qt_sb, wq_sb, xq_sb) if kind == "qb"
                         else (kt_sb, wk_sb, xk_sb))
            g = gen_qkproj(dst, w, x, blk, "vector")
            _gen_ids[g] = (kind, blk)
            return (g, ready, QK_STEP)

        # (generator, est DMA-ready ns, est PE ns per step)
        fills = [
            qk("qb", 1, 23500), qk("kb", 1, 26500),
            vp(0, 21000), vp(1, 21000), vp(2, 21000), vp(3, 21000),
            qk("qb", 2, 33500), qk("kb", 2, 36500),
            vp(4, 30500), vp(5, 30500), vp(6, 30500), vp(7, 30500),
            qk("qb", 3, 42000), qk("kb", 3, 45000),
            vp(8, 39000), vp(9, 39000), vp(10, 39000), vp(11, 39000),
            vp(12, 47500), vp(13, 47500), vp(14, 47500), vp(15, 47500),
        ]
        vp_emitted = [False] * NT       # gen_vproj(t) fully stepped
        qb_emitted = [True, False, False, False]
        kb_emitted = [True, False, False, False]
        state = dict(active=None, pending_ops=[], pe=15000.0, act=15300.0,
                     held_ops=[], steps=0)

        def fill_step(respect_gates=True):
            while True:
                if state["active"] is None:
                    if state["pending_ops"]:
                        state["active"] = (state["pending_ops"].pop(0),
                                           0, OP_STEP)
                    elif fills:
                        if respect_gates and state["pe"] < fills[0][1] - 1500:
                            return False
                        state["active"] = fills.pop(0)
                    else:
                        return False
                g, ready, cost = state["active"]
                try:
                    next(g)
                    state["pe"] = max(state["pe"], ready) + cost
                    state["steps"] += 1
                    return True
                except StopIteration:
                    state["active"] = None
                    kind, idx = _gen_ids.get(g, (None, None))
                    if kind == "vp":
                        vp_emitted[idx] = True
                    elif kind == "qb":
                        qb_emitted[idx] = True
                    elif kind == "kb":
                        kb_emitted[idx] = True

        pairs = [(0, 0), (0, 1), (1, 0), (1, 1),
                 (2, 0), (2, 1), (3, 0), (3, 1)]
        tasks = [(b, hp, skt) for (b, hp) in pairs for skt in range(4 * b + 4)]

        queue = []          # produced-but-unconsumed (b, hp, skt, pt)
        ctx_maps = {}

        def width(b, skt):
            u = skt - 4 * b
            return BLK - 128 * u if u >= 0 else BLK

        def try_consume(limit, drain=False):
            n = 0
            while queue and n < limit:
                b, hp, skt, pt = queue[0]
                if not vp_emitted[skt]:
                    return
                queue.pop(0)
                ctxps = ctx_maps.setdefault((b, hp), {})
                consume(b, hp, skt, pt, ctxps)
                state["pe"] += 2 * (width(b, skt) / 2.4 + 50)
                n += 1
                if skt == 4 * b + 3:    # pair complete
                    normalize_pair(b, hp, ctxps)
                    del ctx_maps[(b, hp)]
                    if hp == 1:
                        for t in range(4 * b, 4 * b + 4):
                            state["held_ops"].append(
                                gen_outproj(t, "scalar" if b == 3
                                            else "vector"))
                    # release out-proj work smoothly, two tiles per pair
                    # completion, always keeping >=2 in reserve as the
                    # tail bridge for the final normalize wait
                    if not drain:
                        while len(state["held_ops"]) > 2 and \
                                len(state["pending_ops"]) < 2:
                            state["pending_ops"].append(
                                state["held_ops"].pop(0))

        prev_act_end = 0.0
        for k, (b, hp, skt) in enumerate(tasks):
            # emission-order invariants (Tile only sees deps on already-
            # emitted instructions):
            #  - the qt/kt writes produce(k) reads must be emitted first
            #  - pt ring: the consume of the slot produce(k) reuses too
            while not (qb_emitted[b] and kb_emitted[skt // 4]):
                fill_step(respect_gates=False)
            while len(queue) >= 11:
                n0 = len(queue)
                try_consume(2)
                if len(queue) == n0:
                    fill_step(respect_gates=False)
            w = width(b, skt)
            # sc pool (2 bufs): produce(k) waits until exp(k-2) done
            state["pe"] = max(state["pe"], prev_act_end)
            pt = produce(b, hp, skt)
            state["pe"] += w / 2.4 + 80
            act_start = max(state["act"], state["pe"])
            new_act_end = act_start + (2 * w + 352) / 1.2
            prev_act_end = state["act"]
            state["act"] = new_act_end
            queue.append((b, hp, skt, pt))
            try_consume(2)
            # fill while the PE is ahead of the exp stream, with a minimum
            # progress floor so fills never pile up into forced blocks
            while state["pe"] + 600 < state["act"]:
                if not fill_step():
                    break
            burst = 0
            while k >= 5 and burst < 4 and \
                    state["steps"] < int(2.6 * (k - 4)):
                if not fill_step():
                    break
                burst += 1

        # drain: the held bridge out-projs interleave with the remaining
        # consumes so the PE has work while the exp tail and the last
        # normalize chain complete; OP12-15 (emitted at (3,1) completion)
        # land behind them in the queue
        state["pending_ops"].extend(state["held_ops"])
        state["held_ops"] = []
        while queue:
            try_consume(2, drain=True)
            fill_step(respect_gates=False)
        state["pending_ops"].extend(state["held_ops"])
        state["held_ops"] = []
        while fill_step(respect_gates=False):
            pass
        if dbg:
            nc.sync.dma_start(dbg["qt"][:], qt_sb[:])
            nc.sync.dma_start(dbg["kt"][:], kt_sb[:])
            nc.sync.dma_start(dbg["v"][:], v_sb[:])
            nc.sync.dma_start(dbg["ctxt"][:], ctxt_sb[:])


def _make_tri():
    i = np.arange(128)[:, None]
    j = np.arange(128)[None, :]
    return (i <= j).astype(NP_DT)


def _prep_core_inputs(inputs, core):
    b = core // 4
    h0 = HPC * (core % 4)
    c0, c1 = h0 * DK, (h0 + HPC) * DK
    f32 = np.float32

    def t_chunks(x):  # [S, D] -> [NB, 128, NCH, BLK] (block-major x^T)
        xt = np.asarray(x, f32).T.reshape(NCH, 128, NB, BLK)
        return np.ascontiguousarray(xt.transpose(2, 1, 0, 3)).astype(NP_DT)

    return {
        "xq_t": t_chunks(inputs["input_Q"][b]),
        "xk_t": t_chunks(inputs["input_K"][b]),
        "xv_t": t_chunks(inputs["input_V"][b]),
        "wq": np.ascontiguousarray(np.asarray(inputs["W_Q"], f32)[:, c0:c1].reshape(NCH, 128, HPC * DK).transpose(1, 0, 2)).astype(NP_DT),
        "wk": np.ascontiguousarray(np.asarray(inputs["W_K"], f32)[:, c0:c1].reshape(NCH, 128, HPC * DK).transpose(1, 0, 2)).astype(NP_DT),
        "wv": np.ascontiguousarray(np.asarray(inputs["W_V"], f32)[:, c0:c1].reshape(NCH, 128, HPC * DK).transpose(1, 0, 2)).astype(NP_DT),
        "wo": np.ascontiguousarray(np.asarray(inputs["W_O"], f32)[c0:c1, :].reshape(2, 128, D).transpose(1, 0, 2)).astype(NP_DT),
        "tri01": _make_tri(),
    }


def get_program():
    global _CACHED_NC
    if _CACHED_NC is None:
        _CACHED_NC = _build_program()
    return _CACHED_NC


def kernel(**inputs):
    global LAST_RESULTS
    nc = get_program()
    in_maps = [_prep_core_inputs(inputs, core) for core in range(N_CORES)]
    res = bass_utils.run_bass_kernel_spmd(
        nc, in_maps, core_ids=list(range(N_CORES)),
        trace=TRACE or bool(int(os.environ.get("BASS_TRACE", "0") or 0)))
    LAST_RESULTS = res
    out = np.zeros((B, S, D), np.float32)
    for core in range(N_CORES):
        out[core // 4] += res.results[core]["out_partial"]
    return out


# revision 35
# speedup vs baseline: 1.1127x; 1.0530x over previous
"""Multi-head attention (B=2, S=2048, D=1024, H=16, dk=dv=64) on 8 trn2 cores.

Sharding: (batch, head-quad) -> core.  Core i handles batch i//4 and the 4
heads [4*(i%4), 4*(i%4)+4).  Each core computes its partial output
context_h @ W_O[h-slice] summed over its 4 heads; the host sums the 4
partials per batch (the "all-reduce" of the row-sharded output projection).

v2 schedule (vs the 205us baseline): the kernel is a single interleaved
stream built around keeping ScalarE's exp pipeline (the serial softmax
resource, ~1.1us per 128x1024 tile) and the PE dense simultaneously:

  - DMA: need-ordered 1MB column-block descriptors (tri, wk, xk[q0], wq,
    xq[q0], wv, xv[q0], xq[q1], xk[q1], wo, xv[q1], xq[q2], xk[q2], ...)
    so K/Q proj block 0 and the first scores/exp start ~15us earlier.
  - PSUM: sc 2x2 banks (scores), ctx 2x1 (A@V accumulators, live per pair),
    fill 2x1 (QKV proj + out-proj groups) -- so "fill" matmuls weave into
    the PE queue mid-pair instead of only at pair boundaries.
  - Causal column restriction: diagonal tasks compute scores/exp/A@V only
    for q >= 128*u (the visible columns); the mask shrinks to one 128x128
    triangle multiply per head (DVE 2x mode) instead of 128x1024.
  - Fills (K/Q proj blocks 1-3, V proj tiles, lagged out-proj tiles) are
    generators stepped ~2x per task between produce/consume so the PE never
    idles long enough to re-throttle (HAM) and ACT never starves.
  - Normalize: one merged [65,512] PSUM->SBUF copy (sums row + ctx rows)
    releases the ctx bank fast; reciprocal/broadcast/multiply off-path.

All matmuls bf16 in / fp32 accumulate; scores pairs run concurrently on
disjoint 64-row PE groups; A@V uses the ones-column trick so the softmax
denominators fall out of the same matmul.
"""

import os
import numpy as np
import ml_dtypes

import concourse.bacc as bacc
import concourse.tile as tile
import concourse.mybir as mybir
import concourse.bass_utils as bass_utils
from concourse.bass import ds

B, S, D, H, DK = 2, 2048, 1024, 16, 64
N_CORES = 8
HPC = 4            # heads per core
NCH = 8            # d-model chunks of 128
NB = 4             # query blocks of 512
BLK = 512
NT = 16            # s tiles of 128
VW = DK + 1        # V columns per head incl. ones column

DT = mybir.dt.bfloat16
NP_DT = ml_dtypes.bfloat16
F32 = mybir.dt.float32

TRACE = False
LAST_RESULTS = None

_CACHED_NC = None


def _build_program():
    nc = bacc.Bacc("TRN2", target_bir_lowering=False, debug=False,
                   enable_asserts=False, num_devices=N_CORES)

    # block-major: [q-block, partition, chunk, s-within-block] so one clean
    # 2D descriptor loads everything a 512-column proj block needs
    xq_d = nc.dram_tensor("xq_t", [NB, 128, NCH, BLK], DT, kind="ExternalInput")
    xk_d = nc.dram_tensor("xk_t", [NB, 128, NCH, BLK], DT, kind="ExternalInput")
    xv_d = nc.dram_tensor("xv_t", [NB, 128, NCH, BLK], DT, kind="ExternalInput")
    wq_d = nc.dram_tensor("wq", [128, NCH, HPC * DK], DT, kind="ExternalInput")
    wk_d = nc.dram_tensor("wk", [128, NCH, HPC * DK], DT, kind="ExternalInput")
    wv_d = nc.dram_tensor("wv", [128, NCH, HPC * DK], DT, kind="ExternalInput")
    wo_d = nc.dram_tensor("wo", [128, 2, D], DT, kind="ExternalInput")
    tri_d = nc.dram_tensor("tri01", [128, 128], DT, kind="ExternalInput")
    out_d = nc.dram_tensor("out_partial", [S, D], F32, kind="ExternalOutput")
    dbg = {}
    if os.environ.get("KDBG"):
        dbg["qt"] = nc.dram_tensor("qt_dump", [128, 2, S], DT, kind="ExternalOutput")
        dbg["kt"] = nc.dram_tensor("kt_dump", [128, 2, S], DT, kind="ExternalOutput")
        dbg["v"] = nc.dram_tensor("v_dump", [128, NT, HPC * VW], DT, kind="ExternalOutput")
        dbg["ctxt"] = nc.dram_tensor("ctxt_dump", [128, 2, S], DT, kind="ExternalOutput")

    with tile.TileContext(nc) as tc:
        _body(tc, xq_d, xk_d, xv_d, wq_d, wk_d, wv_d, wo_d, tri_d, out_d, dbg)
    nc.compile()
    return nc


def _body(tc, xq_d, xk_d, xv_d, wq_d, wk_d, wv_d, wo_d, tri_d, out_d, dbg=None):
    nc = tc.nc
    EXP = mybir.ActivationFunctionType.Exp
    CPY = mybir.ActivationFunctionType.Copy
    MUL = mybir.AluOpType.mult

    with (
        tc.tile_pool(name="consts", bufs=1) as consts,
        tc.tile_pool(name="persist", bufs=1) as persist,
        tc.tile_pool(name="xbufs", bufs=1) as xbufs,
        tc.tile_pool(name="pt", bufs=14) as pt_pool,
        tc.tile_pool(name="raw", bufs=3) as raw_pool,
        tc.tile_pool(name="small", bufs=2) as small,
        tc.tile_pool(name="osb", bufs=2) as ob_pool,
        tc.tile_pool(name="psum_sc", bufs=2, space="PSUM") as sc_pool,
        tc.tile_pool(name="psum_ctx", bufs=2, space="PSUM") as ctx_pool,
        tc.tile_pool(name="psum_fill", bufs=2, space="PSUM") as fill_pool,
    ):
        # ---- constants / persistent activations ----
        wq_sb = consts.tile([128, NCH, HPC * DK], DT)
        wk_sb = consts.tile([128, NCH, HPC * DK], DT)
        wv_sb = consts.tile([128, NCH, HPC * DK], DT)
        wo_sb = consts.tile([128, 2, D], DT)
        tri_sb = consts.tile([128, 128], DT)

        qt_sb = persist.tile([128, 2, S], DT)         # Q^T, pair-major
        kt_sb = persist.tile([128, 2, S], DT)         # K^T
        v_sb = persist.tile([128, NT, HPC * VW], DT)  # V + ones cols
        ctxt_sb = persist.tile([128, 2, S], DT)       # context^T

        xq_sb = xbufs.tile([128, NB, NCH, BLK], DT)
        xk_sb = xbufs.tile([128, NB, NCH, BLK], DT)
        xv_sb = xbufs.tile([128, NB, NCH, BLK], DT)

        # scalar-engine exp-table warmup (runs during the input DMAs)
        scr = small.tile([1, 16], F32, name="scr", tag="scr")
        scr2 = small.tile([1, 16], DT, name="scr2", tag="scr")
        nc.vector.memset(scr[:], 0.0)
        nc.scalar.activation(scr2[:], scr[:], EXP, scale=1.0)

        for hh in range(HPC):
            nc.vector.memset(v_sb[:, :, hh * VW + DK: hh * VW + DK + 1], 1.0)

        # ---- input DMAs: need-ordered 1MB block descriptors ----
        def ld_x(sb, dr, q):
            nc.sync.dma_start(sb[:, q], dr[q])

        # K path on the sync queue, Q path on the gpsimd queue: descriptor
        # issue is ~1.5us each and serializes per queue, so split it
        nc.sync.dma_start(wk_sb[:], wk_d[:])
        nc.sync.dma_start(xk_sb[:, 0, 0:4], xk_d[0][:, 0:4])
        nc.sync.dma_start(xk_sb[:, 0, 4:8], xk_d[0][:, 4:8])
        nc.gpsimd.dma_start(wq_sb[:], wq_d[:])
        nc.gpsimd.dma_start(xq_sb[:, 0, 0:4], xq_d[0][:, 0:4])
        nc.gpsimd.dma_start(xq_sb[:, 0, 4:8], xq_d[0][:, 4:8])
        nc.gpsimd.dma_start(tri_sb[:], tri_d[:])
        nc.sync.dma_start(wv_sb[:], wv_d[:])
        ld_x(xv_sb, xv_d, 0)
        ld_x(xq_sb, xq_d, 1)
        ld_x(xk_sb, xk_d, 1)
        nc.sync.dma_start(wo_sb[:], wo_d[:])
        ld_x(xv_sb, xv_d, 1)
        ld_x(xq_sb, xq_d, 2)
        ld_x(xk_sb, xk_d, 2)
        ld_x(xv_sb, xv_d, 2)
        ld_x(xq_sb, xq_d, 3)
        ld_x(xk_sb, xk_d, 3)
        ld_x(xv_sb, xv_d, 3)

        # ---- fill generators (each yield ~= 2 N=512-class matmuls) ----
        def gen_qkproj(dst, w_sb, x_sb, blk, copy_eng):
            ps = [fill_pool.tile([128, BLK], F32, name=f"qk{p}", tag="fill")
                  for p in range(2)]
            for c in range(NCH):
                for p in range(2):
                    nc.tensor.matmul(
                        ps[p][:],
                        lhsT=w_sb[:, c, ds(128 * p, 128)],
                        rhs=x_sb[:, blk, c, :],
                        start=(c == 0), stop=(c == NCH - 1))
                if c % 2 == 1 and c < NCH - 1:
                    yield
            for p in range(2):
                dstp = dst[:, p, ds(BLK * blk, BLK)]
                if copy_eng == "scalar":
                    nc.scalar.activation(dstp, ps[p][:], CPY)
                else:
                    nc.vector.tensor_copy(dstp, ps[p][:])
            yield

        def gen_vproj(t):
            ps = fill_pool.tile([128, HPC * DK], F32, name="vps", tag="fill")
            for c in range(NCH):
                nc.tensor.matmul(
                    ps[:],
                    lhsT=xv_sb[:, t // 4, c, ds(128 * (t % 4), 128)],
                    rhs=wv_sb[:, c, :],
                    start=(c == 0), stop=(c == NCH - 1))
                if c in (2, 5):
                    yield
            dst = v_sb[:, t, :].rearrange(
                "p (hh e) -> p hh e", hh=HPC)[:, :, 0:DK]
            nc.vector.tensor_copy(dst, ps[:].rearrange(
                "p (hh e) -> p hh e", hh=HPC))
            yield

        def gen_outproj(t, copy_eng="vector"):
            pp = [fill_pool.tile([128, BLK], F32, name=f"pp{nb}", tag="fill")
                  for nb in range(2)]
            for cc in range(2):
                for nb in range(2):
                    nc.tensor.matmul(
                        pp[nb][:],
                        lhsT=ctxt_sb[:, cc, ds(128 * t, 128)],
                        rhs=wo_sb[:, cc, ds(512 * nb, 512)],
                        start=(cc == 0), stop=(cc == 1))
                yield
            ob = ob_pool.tile([128, D], F32, name="ob", tag="ob")
            for nb in range(2):
                dst = ob[:, ds(512 * nb, 512)]
                if copy_eng == "scalar":
                    nc.scalar.activation(dst, pp[nb][:], CPY)
                else:
                    nc.vector.tensor_copy(dst, pp[nb][:])
            nc.sync.dma_start(out_d[ds(128 * t, 128), :], ob[:])
            yield

        # ---- attention stream ops ----
        def produce(b, hp, skt):
            u = skt - 4 * b
            qlo = 128 * u if u >= 0 else 0
            w = BLK - qlo
            sc = sc_pool.tile([128, 2, BLK], F32, name="sc", tag="sc")
            for h2 in range(2):
                nc.tensor.matmul(
                    sc[:, h2, qlo:BLK],
                    lhsT=kt_sb[ds(64 * h2, 64), hp, ds(128 * skt, 128)],
                    rhs=qt_sb[ds(64 * h2, 64), hp,
                              ds(BLK * b + qlo, w)],
                    start=True, stop=True)
            pt = pt_pool.tile([128, 2, BLK], DT, name="pt", tag="pt")
            nc.scalar.activation(pt[:, :, qlo:BLK], sc[:, :, qlo:BLK],
                                 EXP, scale=0.125)
            if u >= 0:
                for h2 in range(2):
                    nc.vector.tensor_tensor(
                        pt[:, h2, qlo:qlo + 128],
                        pt[:, h2, qlo:qlo + 128], tri_sb[:], MUL)
            return pt

        def consume(b, hp, skt, pt, ctxps):
            u = skt - 4 * b
            qlo = 128 * u if u >= 0 else 0
            last = 4 * b + 3
            for h2 in range(2):
                h = 2 * hp + h2
                if h not in ctxps:
                    ctxps[h] = ctx_pool.tile(
                        [128, BLK], F32, name=f"ctx{h2}", tag="ctx")
                nc.tensor.matmul(
                    ctxps[h][0:VW, qlo:BLK],
                    lhsT=v_sb[:, skt, ds(h * VW, VW)],
                    rhs=pt[:, h2, qlo:BLK],
                    start=(skt == 0), stop=(skt == last))

        def normalize_pair(b, hp, ctxps):
            # DVE order: sums+recip first so both gpsimd broadcasts start
            # early; raw copies and multiplies follow (shortest critical path
            # to releasing ctxt for the out-projection).
            sums, r, raw, bc = {}, {}, {}, {}
            for h2 in range(2):
                h = 2 * hp + h2
                sums[h2] = small.tile([1, BLK], F32, name="sums", tag="sums")
                nc.vector.tensor_copy(sums[h2][:], ctxps[h][ds(DK, 1), :])
                r[h2] = small.tile([1, BLK], F32, name="r", tag="r")
                nc.vector.reciprocal_approx_fast(out=r[h2][:], in_=sums[h2][:])
                bc[h2] = small.tile([64, BLK], F32, name="bc", tag="bc")
                nc.gpsimd.partition_broadcast(bc[h2][:], r[h2][:])
            for h2 in range(2):
                h = 2 * hp + h2
                raw[h2] = raw_pool.tile([DK, BLK], F32, name="raw", tag="raw")
                nc.vector.tensor_copy(raw[h2][:], ctxps[h][0:DK, :])
            for h2 in range(2):
                nc.vector.tensor_tensor(
                    ctxt_sb[ds(64 * h2, 64), hp, ds(BLK * b, BLK)],
                    raw[h2][:], bc[h2][:], MUL)

        # ---- the interleaved schedule (model-paced) ----
        # K/Q proj block 0 first (scalar-engine copies: ACT is idle here)
        for _ in gen_qkproj(kt_sb, wk_sb, xk_sb, 0, "scalar"):
            pass
        for _ in gen_qkproj(qt_sb, wq_sb, xq_sb, 0, "scalar"):
            pass

        # build-time cost model (ns) to pace fills between produce/consume
        QK_STEP, VP_STEP, OP_STEP = 520, 340, 520
        _gen_ids = {}

        def vp(t, ready):
            g = gen_vproj(t)
            _gen_ids[g] = ("vp", t)
            return (g, ready, VP_STEP)

        def qk(kind, blk, ready):
            dst, w, x = ((qt_sb, wq_sb, xq_sb) if kind == "qb"
                         else (kt_sb, wk_sb, xk_sb))
            g = gen_qkproj(dst, w, x, blk, "vector")
            _gen_ids[g] = (kind, blk)
            return (g, ready, QK_STEP)

        # (generator, est DMA-ready ns, est PE ns per step)
        fills = [
            qk("qb", 1, 23500), qk("kb", 1, 26500),
            vp(0, 21000), vp(1, 21000), vp(2, 21000), vp(3, 21000),
            qk("qb", 2, 33500), qk("kb", 2, 36500),
            vp(4, 30500), vp(5, 30500), vp(6, 30500), vp(7, 30500),
            qk("qb", 3, 42000), qk("kb", 3, 45000),
            vp(8, 39000), vp(9, 39000), vp(10, 39000), vp(11, 39000),
            vp(12, 47500), vp(13, 47500), vp(14, 47500), vp(15, 47500),
        ]
        vp_emitted = [False] * NT       # gen_vproj(t) fully stepped
        qb_emitted = [True, False, False, False]
        kb_emitted = [True, False, False, False]
        state = dict(active=None, pending_ops=[], pe=15000.0, act=15300.0,
                     held_ops=[], steps=0)

        def fill_step(respect_gates=True):
            while True:
                if state["active"] is None:
                    if state["pending_ops"]:
                        state["active"] = (state["pending_ops"].pop(0),
                                           0, OP_STEP)
                    elif fills:
                        if respect_gates and state["pe"] < fills[0][1] - 1500:
                            return False
                        state["active"] = fills.pop(0)
                    else:
                        return False
                g, ready, cost = state["active"]
                try:
                    next(g)
                    state["pe"] = max(state["pe"], ready) + cost
                    state["steps"] += 1
                    return True
                except StopIteration:
                    state["active"] = None
                    kind, idx = _gen_ids.get(g, (None, None))
                    if kind == "vp":
                        vp_emitted[idx] = True
                    elif kind == "qb":
                        qb_emitted[idx] = True
                    elif kind == "kb":
                        kb_emitted[idx] = True

        pairs = [(0, 0), (0, 1), (1, 0), (1, 1),
                 (2, 0), (2, 1), (3, 0), (3, 1)]
        tasks = [(b, hp, skt) for (b, hp) in pairs for skt in range(4 * b + 4)]

        queue = []          # produced-but-unconsumed (b, hp, skt, pt)
        ctx_maps = {}

        def width(b, skt):
            u = skt - 4 * b
            return BLK - 128 * u if u >= 0 else BLK

        def try_consume(limit, drain=False):
            n = 0
            while queue and n < limit:
                b, hp, skt, pt = queue[0]
                if not vp_emitted[skt]:
                    return
                queue.pop(0)
                ctxps = ctx_maps.setdefault((b, hp), {})
                consume(b, hp, skt, pt, ctxps)
                state["pe"] += 2 * (width(b, skt) / 2.4 + 50)
                n += 1
                if skt == 4 * b + 3:    # pair complete
                    normalize_pair(b, hp, ctxps)
                    del ctx_maps[(b, hp)]
                    if hp == 1:
                        for t in range(4 * b, 4 * b + 4):
                            state["held_ops"].append(
                                gen_outproj(t, "scalar" if b == 3
                                            else "vector"))
                    # release out-proj work smoothly, two tiles per pair
                    # completion, always keeping >=4 in reserve as the
                    # tail bridge for the final normalize wait
                    if not drain:
                        while len(state["held_ops"]) > 4 and \
                                len(state["pending_ops"]) < 2:
                            state["pending_ops"].append(
                                state["held_ops"].pop(0))

        prev_act_end = 0.0
        for k, (b, hp, skt) in enumerate(tasks):
            # emission-order invariants (Tile only sees deps on already-
            # emitted instructions):
            #  - the qt/kt writes produce(k) reads must be emitted first
            #  - pt ring: the consume of the slot produce(k) reuses too
            while not (qb_emitted[b] and kb_emitted[skt // 4]):
                fill_step(respect_gates=False)
            while len(queue) >= 11:
                n0 = len(queue)
                try_consume(2)
                if len(queue) == n0:
                    fill_step(respect_gates=False)
            w = width(b, skt)
            # sc pool (2 bufs): produce(k) waits until exp(k-2) done
            state["pe"] = max(state["pe"], prev_act_end)
            pt = produce(b, hp, skt)
            state["pe"] += w / 2.4 + 80
            act_start = max(state["act"], state["pe"])
            new_act_end = act_start + (2 * w + 352) / 1.2
            prev_act_end = state["act"]
            state["act"] = new_act_end
            queue.append((b, hp, skt, pt))
            try_consume(2)
            # fill while the PE is ahead of the exp stream, with a minimum
            # progress floor so fills never pile up into forced blocks
            while state["pe"] + 600 < state["act"]:
                if not fill_step():
                    break
            burst = 0
            while k >= 5 and burst < 4 and \
                    state["steps"] < min(int(2.6 * (k - 4)), 92):
                if not fill_step():
                    break
                burst += 1

        # drain: the held bridge out-projs interleave with the remaining
        # consumes so the PE has work while the exp tail and the last
        # normalize chain complete; OP12-15 (emitted at (3,1) completion)
        # land behind them in the queue
        state["pending_ops"].extend(state["held_ops"])
        state["held_ops"] = []
        while queue:
            try_consume(2, drain=True)
            fill_step(respect_gates=False)
        state["pending_ops"].extend(state["held_ops"])
        state["held_ops"] = []
        while fill_step(respect_gates=False):
            pass
        if dbg:
            nc.sync.dma_start(dbg["qt"][:], qt_sb[:])
            nc.sync.dma_start(dbg["kt"][:], kt_sb[:])
            nc.sync.dma_start(dbg["v"][:], v_sb[:])
            nc.sync.dma_start(dbg["ctxt"][:], ctxt_sb[:])


def _make_tri():
    i = np.arange(128)[:, None]
    j = np.arange(128)[None, :]
    return (i <= j).astype(NP_DT)


def _prep_core_inputs(inputs, core):
    b = core // 4
    h0 = HPC * (core % 4)
    c0, c1 = h0 * DK, (h0 + HPC) * DK
    f32 = np.float32

    def t_chunks(x):  # [S, D] -> [NB, 128, NCH, BLK] (block-major x^T)
        xt = np.asarray(x, f32).T.reshape(NCH, 128, NB, BLK)
        return np.ascontiguousarray(xt.transpose(2, 1, 0, 3)).astype(NP_DT)

    return {
        "xq_t": t_chunks(inputs["input_Q"][b]),
        "xk_t": t_chunks(inputs["input_K"][b]),
        "xv_t": t_chunks(inputs["input_V"][b]),
        "wq": np.ascontiguousarray(np.asarray(inputs["W_Q"], f32)[:, c0:c1].reshape(NCH, 128, HPC * DK).transpose(1, 0, 2)).astype(NP_DT),
        "wk": np.ascontiguousarray(np.asarray(inputs["W_K"], f32)[:, c0:c1].reshape(NCH, 128, HPC * DK).transpose(1, 0, 2)).astype(NP_DT),
        "wv": np.ascontiguousarray(np.asarray(inputs["W_V"], f32)[:, c0:c1].reshape(NCH, 128, HPC * DK).transpose(1, 0, 2)).astype(NP_DT),
        "wo": np.ascontiguousarray(np.asarray(inputs["W_O"], f32)[c0:c1, :].reshape(2, 128, D).transpose(1, 0, 2)).astype(NP_DT),
        "tri01": _make_tri(),
    }


def get_program():
    global _CACHED_NC
    if _CACHED_NC is None:
        _CACHED_NC = _build_program()
    return _CACHED_NC


def kernel(**inputs):
    global LAST_RESULTS
    nc = get_program()
    in_maps = [_prep_core_inputs(inputs, core) for core in range(N_CORES)]
    res = bass_utils.run_bass_kernel_spmd(
        nc, in_maps, core_ids=list(range(N_CORES)),
        trace=TRACE or bool(int(os.environ.get("BASS_TRACE", "0") or 0)))
    LAST_RESULTS = res
    out = np.zeros((B, S, D), np.float32)
    for core in range(N_CORES):
        out[core // 4] += res.results[core]["out_partial"]
    return out
